# revision 12
# baseline (speedup 1.0000x reference)
"""AggregatedContrastiveLoss on 8 Trainium2 NeuronCores.

Strategy (data-parallel over the N=2M points dimension):
  - Each of 8 cores streams a ~250k-point shard of pred (as fp16) from HBM.
  - Per 128-point group, TensorE accumulates predT @ onehot[128, 304] into
    PSUM [128d, 304] — the per-(class, group) feature sums for group A
    (cols 0..149) and group B (cols 152..301), where the one-hot selection
    matrix comes from a host-packed key (key = seg + 152*group, or an
    out-of-range value for masked/overlap points).
  - One-hot groups are produced by three sources in parallel, balanced so
    DMA, VectorE and ScalarE all saturate together:
      * k_pre groups/chunk: precomputed fp16 one-hots DMAd from HBM
      * middle groups: VectorE tensor_scalar is_equal vs an iota row
      * k_act groups/chunk: ScalarE relu(1 - |iota - key|) (exact)
  - Host reduces the 8 partial [128,304] outputs, computes per-class counts
    from the (tiny) int arrays, and runs the [150,128]-level normalize +
    [150,150] InfoNCE finalize in float64.
The `target` input is unused by the loss math and never transferred.
"""
import numpy as np

import concourse.bacc as bacc
import concourse.mybir as mybir
import concourse.tile as tile
from concourse.bass_utils import run_bass_kernel_spmd

F32 = mybir.dt.float32
F16 = mybir.dt.float16
AF = mybir.ActivationFunctionType

N = 2_000_000
D = 128
C = 150
TEMPERATURE = 0.2
LOSS_WEIGHT = 1.0

N_CORES = 8
OWN = N // N_CORES            # 250_000 points owned per core
SHARD = 250_112               # 1954 groups of 128 (>= OWN, multiple of 128)
N_TILES = SHARD // 128        # 1954
CHUNK_TILES = 64              # 8192 points (2 MB fp16) per pred DMA
K_PRE = 12                    # groups/chunk with precomputed one-hot (DMA)
K_ACT = 9                     # groups/chunk built on ScalarE
W = 304                       # one-hot width (A: 0..149, B: 152..301)
BOFF = 152
INVALID = 1000.0

_STARTS = [min(i * OWN, N - SHARD) for i in range(N_CORES)]


def _plan_chunks():
    chunks = []
    rem = N_TILES
    first = [8, 8, 16, 32]    # priming chunks fill the pipeline quickly
    for ct in first:
        chunks.append(ct)
        rem -= ct
    while rem > 0:
        chunks.append(min(CHUNK_TILES, rem))
        rem -= chunks[-1]
    return [
        (ct, (ct * K_PRE) // CHUNK_TILES, (ct * K_ACT) // CHUNK_TILES)
        for ct in chunks
    ]


_CHUNKS = _plan_chunks()
_PRE_TOT = sum(kp for _, kp, _ in _CHUNKS)


def _interleave(ct, kp, ka):
    """Spread the three one-hot sources evenly across a chunk's groups so
    PE's in-order PSUM consumption sees a homogeneous production mix."""
    srcs = []
    acc_p = acc_a = 0.0
    for j in range(ct):
        acc_p += kp / ct
        acc_a += ka / ct
        if acc_p >= 1.0:
            srcs.append("pre")
            acc_p -= 1.0
        elif acc_a >= 1.0:
            srcs.append("act")
            acc_a -= 1.0
        else:
            srcs.append("dve")
    # fix rounding drift
    while srcs.count("pre") < kp:
        srcs[srcs.index("dve")] = "pre"
    while srcs.count("act") < ka:
        srcs[len(srcs) - 1 - srcs[::-1].index("dve")] = "act"
    assert srcs.count("pre") == kp and srcs.count("act") == ka
    return srcs


def _build_nc():
    nc = bacc.Bacc(
        "TRN2", target_bir_lowering=False, debug=False, num_devices=N_CORES
    )
    pred_d = nc.dram_tensor("pred", [SHARD, D], F16, kind="ExternalInput")
    key_d = nc.dram_tensor("key", [128, N_TILES], F32, kind="ExternalInput")
    nkey_d = nc.dram_tensor("nkey", [128, N_TILES], F32, kind="ExternalInput")
    iota_d = nc.dram_tensor("iota", [128, W], F16, kind="ExternalInput")
    ohpre_d = nc.dram_tensor(
        "ohpre", [128, _PRE_TOT * W], F16, kind="ExternalInput"
    )
    out_d = nc.dram_tensor("out", [128, W], F32, kind="ExternalOutput")

    with tile.TileContext(nc) as tc:
        with (
            tc.tile_pool(name="io", bufs=5) as pio,
            tc.tile_pool(name="pre", bufs=4) as ppre,
            tc.tile_pool(name="oh", bufs=24) as poh,
            tc.tile_pool(name="tmp", bufs=3) as ptmp,
            tc.tile_pool(name="const", bufs=1) as pconst,
            tc.tile_pool(name="psum", bufs=1, space="PSUM") as pps,
        ):
            iota_sb = pconst.tile([128, W], F16)
            nc.sync.dma_start(iota_sb[:], iota_d[:])
            key_sb = pconst.tile([128, N_TILES], F32)
            nc.sync.dma_start(key_sb[:], key_d[:])
            nkey_sb = pconst.tile([128, N_TILES], F32)
            nc.sync.dma_start(nkey_sb[:], nkey_d[:])
            acc = pps.tile([128, W], F32)

            t_idx = 0
            row = 0
            pre_off = 0
            for ct, kp, ka in _CHUNKS:
                npts = ct * 128
                if kp > 0:
                    pre = ppre.tile([128, kp * W], F16, tag="pre")
                    nc.scalar.dma_start(
                        pre[:], ohpre_d[:, pre_off * W : (pre_off + kp) * W]
                    )
                ph = pio.tile([128, npts], F16, tag="ph")
                src = pred_d[row : row + npts, :].rearrange(
                    "(p j) d -> p (j d)", p=128
                )
                nc.sync.dma_start(ph[:], src)
                srcs = _interleave(ct, kp, ka)
                pre_used = 0
                for j in range(ct):
                    if srcs[j] == "pre":
                        rhs = pre[:, pre_used * W : (pre_used + 1) * W]
                        pre_used += 1
                    elif srcs[j] == "act":
                        # ScalarE path: oh = relu(1 - |iota - key|), exact
                        oh = poh.tile([128, W], F16)
                        tmp = ptmp.tile([128, W], F16)
                        nc.scalar.activation(
                            tmp[:], iota_sb[:], AF.Abs,
                            bias=nkey_sb[:, t_idx : t_idx + 1], scale=1.0,
                        )
                        nc.scalar.activation(
                            oh[:], tmp[:], AF.Relu, bias=1.0, scale=-1.0,
                        )
                        rhs = oh[:]
                    else:
                        oh = poh.tile([128, W], F16)
                        nc.vector.tensor_scalar(
                            oh[:],
                            iota_sb[:],
                            key_sb[:, t_idx : t_idx + 1],
                            None,
                            mybir.AluOpType.is_equal,
                        )
                        rhs = oh[:]
                    nc.tensor.matmul(
                        acc[:],
                        ph[:, j * 128 : (j + 1) * 128],
                        rhs,
                        start=(t_idx == 0),
                        stop=(t_idx == N_TILES - 1),
                    )
                    t_idx += 1
                pre_off += kp
                row += npts
            out_sb = pconst.tile([128, W], F32)
            nc.vector.tensor_copy(out_sb[:], acc[:])
            nc.sync.dma_start(out_d[:], out_sb[:])
    nc.compile()
    return nc


_NC = None


def _get_nc():
    global _NC
    if _NC is None:
        _NC = _build_nc()
    return _NC


def _key_layout(key_flat: np.ndarray) -> np.ndarray:
    """[SHARD] f32 -> [128, N_TILES] f32 matching the kernel's point order:
    within a chunk of `ct` groups starting at flat row `row`, partition p,
    column j holds point row + p*ct + j."""
    cols = []
    row = 0
    for ct, _, _ in _CHUNKS:
        cols.append(key_flat[row : row + ct * 128].reshape(128, ct))
        row += ct * 128
    return np.ascontiguousarray(np.concatenate(cols, axis=1))


_PRE_IDX = []
_t = 0
for _ct, _kp, _ka in _CHUNKS:
    _srcs = _interleave(_ct, _kp, _ka)
    _PRE_IDX.extend(_t + _j for _j in range(_ct) if _srcs[_j] == "pre")
    _t += _ct


def _host_ohpre(key2d: np.ndarray) -> np.ndarray:
    sel = key2d[:, _PRE_IDX]                     # [128, PRE_TOT]
    oh = sel[:, :, None] == np.arange(W, dtype=np.float32)[None, None, :]
    return np.ascontiguousarray(
        oh.astype(np.float16).reshape(128, _PRE_TOT * W)
    )


def _prep_in_maps(pred, key_full):
    iota = np.tile(np.arange(W, dtype=np.float16), (128, 1))
    in_maps = []
    for i in range(N_CORES):
        s = _STARTS[i]
        k = key_full[s : s + SHARD].copy()
        own_lo, own_hi = i * OWN, (i + 1) * OWN
        gidx = np.arange(s, s + SHARD)
        k[(gidx < own_lo) | (gidx >= own_hi)] = INVALID
        k2 = _key_layout(k)
        in_maps.append(
            {
                "pred": np.ascontiguousarray(
                    pred[s : s + SHARD], dtype=np.float16
                ),
                "key": k2,
                "nkey": -k2,
                "iota": iota,
                "ohpre": _host_ohpre(k2),
            }
        )
    return in_maps


def _make_keys(seg, grp, vm):
    valid = (vm > 0) & (seg != -1)
    segc = np.clip(seg, 0, C - 1)
    in_group = (grp == 0) | (grp == 1)
    key_full = np.where(
        valid & in_group, segc + BOFF * grp, int(INVALID)
    ).astype(np.float32)
    return key_full, valid, segc


def kernel(pred, target, valid_feat_mask, segment, group_assign):
    pred = np.asarray(pred, dtype=np.float32)
    seg = np.asarray(segment).astype(np.int64)
    grp = np.asarray(group_assign).astype(np.int64)
    vm = np.asarray(valid_feat_mask)

    key_full, valid, segc = _make_keys(seg, grp, vm)
    in_maps = _prep_in_maps(pred, key_full)

    nc = _get_nc()
    res = run_bass_kernel_spmd(nc, in_maps, core_ids=list(range(N_CORES)))

    total = np.zeros((128, W), np.float64)
    for r in res.results:
        total += r["out"].astype(np.float64)
    sum_a = total[:, 0:C].T          # [C, D]
    sum_b = total[:, BOFF : BOFF + C].T

    ga = valid & (grp == 0)
    gb = valid & (grp == 1)
    cnt_a = np.bincount(segc[ga], minlength=C).astype(np.float64)
    cnt_b = np.bincount(segc[gb], minlength=C).astype(np.float64)

    mean_a = sum_a / np.maximum(cnt_a, 1.0)[:, None]
    mean_b = sum_b / np.maximum(cnt_b, 1.0)[:, None]
    a = mean_a / np.linalg.norm(mean_a, axis=1, keepdims=True)
    b = mean_b / np.linalg.norm(mean_b, axis=1, keepdims=True)
    logits = (a @ b.T) / TEMPERATURE
    diag = np.diagonal(logits)

    def lse(x, axis):
        m = x.max(axis=axis)
        return m + np.log(np.exp(x - np.expand_dims(m, axis)).sum(axis=axis))

    loss_a = np.mean(lse(logits, 1) - diag)
    loss_b = np.mean(lse(logits, 0) - diag)
    loss = LOSS_WEIGHT * (loss_a + loss_b) / 2.0
    return np.asarray(loss, dtype=np.float32)


# revision 13
# speedup vs baseline: 1.3001x; 1.3001x over previous
"""AggregatedContrastiveLoss on 8 Trainium2 NeuronCores.

Strategy (data-parallel over the N=2M points dimension):
  - Each of 8 cores streams a ~250k-point shard of pred (as fp16) from HBM.
  - Per 128-point group, TensorE accumulates predT @ onehot[128, 304] into
    PSUM [128d, 304] — the per-(class, group) feature sums for group A
    (cols 0..149) and group B (cols 152..301), where the one-hot selection
    matrix comes from a host-packed key (key = seg + 152*group, or an
    out-of-range value for masked/overlap points).
  - One-hot groups are produced by three sources in parallel, balanced so
    DMA, VectorE and ScalarE all saturate together:
      * k_pre groups/chunk: precomputed fp16 one-hots DMAd from HBM
      * middle groups: VectorE tensor_scalar is_equal vs an iota row
      * k_act groups/chunk: ScalarE relu(1 - |iota - key|) (exact)
  - Host reduces the 8 partial [128,304] outputs, computes per-class counts
    from the (tiny) int arrays, and runs the [150,128]-level normalize +
    [150,150] InfoNCE finalize in float64.
The `target` input is unused by the loss math and never transferred.
"""
import numpy as np

import concourse.bacc as bacc
import concourse.mybir as mybir
import concourse.tile as tile
from concourse.bass_utils import run_bass_kernel_spmd

F32 = mybir.dt.float32
F16 = mybir.dt.float16
AF = mybir.ActivationFunctionType

N = 2_000_000
D = 128
C = 150
TEMPERATURE = 0.2
LOSS_WEIGHT = 1.0

N_CORES = 8
OWN = N // N_CORES            # 250_000 points owned per core
SHARD = 250_112               # 1954 groups of 128 (>= OWN, multiple of 128)
N_TILES = SHARD // 128        # 1954
CHUNK_TILES = 64              # 8192 points (2 MB fp16) per pred DMA
K_PRE = 12                    # groups/chunk with precomputed one-hot (DMA)
K_ACT = 9                     # groups/chunk built on ScalarE
W = 304                       # one-hot width (A: 0..149, B: 152..301)
BOFF = 152
INVALID = 1000.0

_STARTS = [min(i * OWN, N - SHARD) for i in range(N_CORES)]


def _plan_chunks():
    chunks = []
    rem = N_TILES
    first = [8, 8, 16, 32]    # priming chunks fill the pipeline quickly
    for ct in first:
        chunks.append(ct)
        rem -= ct
    while rem > 0:
        chunks.append(min(CHUNK_TILES, rem))
        rem -= chunks[-1]
    return [
        (ct, (ct * K_PRE) // CHUNK_TILES, (ct * K_ACT) // CHUNK_TILES)
        for ct in chunks
    ]


_CHUNKS = _plan_chunks()
_PRE_TOT = sum(kp for _, kp, _ in _CHUNKS)


def _interleave(ct, kp, ka):
    """Spread the three one-hot sources evenly across a chunk's groups so
    PE's in-order PSUM consumption sees a homogeneous production mix."""
    srcs = []
    acc_p = acc_a = 0.0
    for j in range(ct):
        acc_p += kp / ct
        acc_a += ka / ct
        if acc_p >= 1.0:
            srcs.append("pre")
            acc_p -= 1.0
        elif acc_a >= 1.0:
            srcs.append("act")
            acc_a -= 1.0
        else:
            srcs.append("dve")
    # fix rounding drift
    while srcs.count("pre") < kp:
        srcs[srcs.index("dve")] = "pre"
    while srcs.count("act") < ka:
        srcs[len(srcs) - 1 - srcs[::-1].index("dve")] = "act"
    assert srcs.count("pre") == kp and srcs.count("act") == ka
    return srcs


def _build_nc():
    nc = bacc.Bacc(
        "TRN2", target_bir_lowering=False, debug=False, num_devices=N_CORES
    )
    pred_d = nc.dram_tensor("pred", [SHARD, D], F16, kind="ExternalInput")
    key_d = nc.dram_tensor("key", [128, N_TILES], F32, kind="ExternalInput")
    nkey_d = nc.dram_tensor("nkey", [128, N_TILES], F32, kind="ExternalInput")
    iota_d = nc.dram_tensor("iota", [128, W], F16, kind="ExternalInput")
    ohpre_d = nc.dram_tensor(
        "ohpre", [128, _PRE_TOT * W], F16, kind="ExternalInput"
    )
    out_d = nc.dram_tensor("out", [128, W], F32, kind="ExternalOutput")

    with tile.TileContext(nc) as tc:
        with (
            tc.tile_pool(name="io", bufs=5) as pio,
            tc.tile_pool(name="pre", bufs=3) as ppre,
            tc.tile_pool(name="oh", bufs=24) as poh,
            tc.tile_pool(name="tmp", bufs=3) as ptmp,
            tc.tile_pool(name="const", bufs=1) as pconst,
            tc.tile_pool(name="psum", bufs=1, space="PSUM") as pps,
        ):
            iota_sb = pconst.tile([128, W], F16)
            nc.sync.dma_start(iota_sb[:], iota_d[:])
            key_sb = pconst.tile([128, N_TILES], F32)
            nc.sync.dma_start(key_sb[:], key_d[:])
            nkey_sb = pconst.tile([128, N_TILES], F32)
            nc.sync.dma_start(nkey_sb[:], nkey_d[:])
            acc = pps.tile([128, W], F32)

            t_idx = 0
            row = 0
            pre_off = 0
            for ct, kp, ka in _CHUNKS:
                npts = ct * 128
                if kp > 0:
                    pre = ppre.tile([128, kp * W], F16, tag="pre")
                    nc.sync.dma_start(
                        pre[:], ohpre_d[:, pre_off * W : (pre_off + kp) * W]
                    )
                ph = pio.tile([128, npts], F16, tag="ph")
                src = pred_d[row : row + npts, :].rearrange(
                    "(p j) d -> p (j d)", p=128
                )
                nc.sync.dma_start(ph[:], src)
                srcs = _interleave(ct, kp, ka)
                pre_used = 0
                for j in range(ct):
                    if srcs[j] == "pre":
                        rhs = pre[:, pre_used * W : (pre_used + 1) * W]
                        pre_used += 1
                    elif srcs[j] == "act":
                        # ScalarE path: oh = relu(1 - |iota - key|), exact
                        oh = poh.tile([128, W], F16)
                        tmp = ptmp.tile([128, W], F16)
                        nc.scalar.activation(
                            tmp[:], iota_sb[:], AF.Abs,
                            bias=nkey_sb[:, t_idx : t_idx + 1], scale=1.0,
                        )
                        nc.scalar.activation(
                            oh[:], tmp[:], AF.Relu, bias=1.0, scale=-1.0,
                        )
                        rhs = oh[:]
                    else:
                        oh = poh.tile([128, W], F16)
                        nc.vector.tensor_scalar(
                            oh[:],
                            iota_sb[:],
                            key_sb[:, t_idx : t_idx + 1],
                            None,
                            mybir.AluOpType.is_equal,
                        )
                        rhs = oh[:]
                    nc.tensor.matmul(
                        acc[:],
                        ph[:, j * 128 : (j + 1) * 128],
                        rhs,
                        start=(t_idx == 0),
                        stop=(t_idx == N_TILES - 1),
                    )
                    t_idx += 1
                pre_off += kp
                row += npts
            out_sb = pconst.tile([128, W], F32)
            nc.vector.tensor_copy(out_sb[:], acc[:])
            nc.sync.dma_start(out_d[:], out_sb[:])
    nc.compile()
    return nc


_NC = None


def _get_nc():
    global _NC
    if _NC is None:
        _NC = _build_nc()
    return _NC


def _key_layout(key_flat: np.ndarray) -> np.ndarray:
    """[SHARD] f32 -> [128, N_TILES] f32 matching the kernel's point order:
    within a chunk of `ct` groups starting at flat row `row`, partition p,
    column j holds point row + p*ct + j."""
    cols = []
    row = 0
    for ct, _, _ in _CHUNKS:
        cols.append(key_flat[row : row + ct * 128].reshape(128, ct))
        row += ct * 128
    return np.ascontiguousarray(np.concatenate(cols, axis=1))


_PRE_IDX = []
_t = 0
for _ct, _kp, _ka in _CHUNKS:
    _srcs = _interleave(_ct, _kp, _ka)
    _PRE_IDX.extend(_t + _j for _j in range(_ct) if _srcs[_j] == "pre")
    _t += _ct


def _host_ohpre(key2d: np.ndarray) -> np.ndarray:
    sel = key2d[:, _PRE_IDX]                     # [128, PRE_TOT]
    oh = sel[:, :, None] == np.arange(W, dtype=np.float32)[None, None, :]
    return np.ascontiguousarray(
        oh.astype(np.float16).reshape(128, _PRE_TOT * W)
    )


def _prep_in_maps(pred, key_full):
    iota = np.tile(np.arange(W, dtype=np.float16), (128, 1))
    in_maps = []
    for i in range(N_CORES):
        s = _STARTS[i]
        k = key_full[s : s + SHARD].copy()
        own_lo, own_hi = i * OWN, (i + 1) * OWN
        gidx = np.arange(s, s + SHARD)
        k[(gidx < own_lo) | (gidx >= own_hi)] = INVALID
        k2 = _key_layout(k)
        in_maps.append(
            {
                "pred": np.ascontiguousarray(
                    pred[s : s + SHARD], dtype=np.float16
                ),
                "key": k2,
                "nkey": -k2,
                "iota": iota,
                "ohpre": _host_ohpre(k2),
            }
        )
    return in_maps


def _make_keys(seg, grp, vm):
    valid = (vm > 0) & (seg != -1)
    segc = np.clip(seg, 0, C - 1)
    in_group = (grp == 0) | (grp == 1)
    key_full = np.where(
        valid & in_group, segc + BOFF * grp, int(INVALID)
    ).astype(np.float32)
    return key_full, valid, segc


def kernel(pred, target, valid_feat_mask, segment, group_assign):
    pred = np.asarray(pred, dtype=np.float32)
    seg = np.asarray(segment).astype(np.int64)
    grp = np.asarray(group_assign).astype(np.int64)
    vm = np.asarray(valid_feat_mask)

    key_full, valid, segc = _make_keys(seg, grp, vm)
    in_maps = _prep_in_maps(pred, key_full)

    nc = _get_nc()
    res = run_bass_kernel_spmd(nc, in_maps, core_ids=list(range(N_CORES)))

    total = np.zeros((128, W), np.float64)
    for r in res.results:
        total += r["out"].astype(np.float64)
    sum_a = total[:, 0:C].T          # [C, D]
    sum_b = total[:, BOFF : BOFF + C].T

    ga = valid & (grp == 0)
    gb = valid & (grp == 1)
    cnt_a = np.bincount(segc[ga], minlength=C).astype(np.float64)
    cnt_b = np.bincount(segc[gb], minlength=C).astype(np.float64)

    mean_a = sum_a / np.maximum(cnt_a, 1.0)[:, None]
    mean_b = sum_b / np.maximum(cnt_b, 1.0)[:, None]
    a = mean_a / np.linalg.norm(mean_a, axis=1, keepdims=True)
    b = mean_b / np.linalg.norm(mean_b, axis=1, keepdims=True)
    logits = (a @ b.T) / TEMPERATURE
    diag = np.diagonal(logits)

    def lse(x, axis):
        m = x.max(axis=axis)
        return m + np.log(np.exp(x - np.expand_dims(m, axis)).sum(axis=axis))

    loss_a = np.mean(lse(logits, 1) - diag)
    loss_b = np.mean(lse(logits, 0) - diag)
    loss = LOSS_WEIGHT * (loss_a + loss_b) / 2.0
    return np.asarray(loss, dtype=np.float32)


# revision 14
# speedup vs baseline: 1.3745x; 1.0572x over previous
"""AggregatedContrastiveLoss on 8 Trainium2 NeuronCores.

Strategy (data-parallel over the N=2M points dimension):
  - Each of 8 cores streams a ~250k-point shard of pred (as fp16) from HBM.
  - Per 128-point group, TensorE accumulates predT @ onehot[128, 304] into
    PSUM [128d, 304] — the per-(class, group) feature sums for group A
    (cols 0..149) and group B (cols 152..301), where the one-hot selection
    matrix comes from a host-packed key (key = seg + 152*group, or an
    out-of-range value for masked/overlap points).
  - One-hot groups are produced by three sources in parallel, balanced so
    DMA, VectorE and ScalarE all saturate together:
      * k_pre groups/chunk: precomputed fp16 one-hots DMAd from HBM
      * middle groups: VectorE tensor_scalar is_equal vs an iota row
      * k_act groups/chunk: ScalarE relu(1 - |iota - key|) (exact)
  - Host reduces the 8 partial [128,304] outputs, computes per-class counts
    from the (tiny) int arrays, and runs the [150,128]-level normalize +
    [150,150] InfoNCE finalize in float64.
The `target` input is unused by the loss math and never transferred.
"""
import numpy as np

import concourse.bacc as bacc
import concourse.mybir as mybir
import concourse.tile as tile
from concourse.bass_utils import run_bass_kernel_spmd

F32 = mybir.dt.float32
F16 = mybir.dt.float16
F8 = mybir.dt.float8e4
AF = mybir.ActivationFunctionType

N = 2_000_000
D = 128
C = 150
TEMPERATURE = 0.2
LOSS_WEIGHT = 1.0

N_CORES = 8
OWN = N // N_CORES            # 250_000 points owned per core
SHARD = 250_112               # 1954 groups of 128 (>= OWN, multiple of 128)
N_TILES = SHARD // 128        # 1954
CHUNK_TILES = 64              # 8192 points (2 MB fp16) per pred DMA
K_PRE = 17                    # groups/chunk with precomputed one-hot (DMA)
K_ACT = 8                     # groups/chunk built on ScalarE
W = 304                       # one-hot width (A: 0..149, B: 152..301)
BOFF = 152
INVALID = 1000.0

_STARTS = [min(i * OWN, N - SHARD) for i in range(N_CORES)]


def _plan_chunks():
    chunks = []
    rem = N_TILES
    first = [8, 8, 16, 32]    # priming chunks fill the pipeline quickly
    for ct in first:
        chunks.append(ct)
        rem -= ct
    while rem > 0:
        chunks.append(min(CHUNK_TILES, rem))
        rem -= chunks[-1]
    return [
        (ct, (ct * K_PRE) // CHUNK_TILES, (ct * K_ACT) // CHUNK_TILES)
        for ct in chunks
    ]


_CHUNKS = _plan_chunks()
_PRE_TOT = sum(kp for _, kp, _ in _CHUNKS)


def _interleave(ct, kp, ka):
    """Spread the three one-hot sources evenly across a chunk's groups so
    PE's in-order PSUM consumption sees a homogeneous production mix."""
    srcs = []
    acc_p = acc_a = 0.0
    for j in range(ct):
        acc_p += kp / ct
        acc_a += ka / ct
        if acc_p >= 1.0:
            srcs.append("pre")
            acc_p -= 1.0
        elif acc_a >= 1.0:
            srcs.append("act")
            acc_a -= 1.0
        else:
            srcs.append("dve")
    # fix rounding drift
    while srcs.count("pre") < kp:
        srcs[srcs.index("dve")] = "pre"
    while srcs.count("act") < ka:
        srcs[len(srcs) - 1 - srcs[::-1].index("dve")] = "act"
    assert srcs.count("pre") == kp and srcs.count("act") == ka
    return srcs


def _build_nc():
    nc = bacc.Bacc(
        "TRN2", target_bir_lowering=False, debug=False, num_devices=N_CORES
    )
    pred_d = nc.dram_tensor("pred", [SHARD, D], F16, kind="ExternalInput")
    key_d = nc.dram_tensor("key", [128, N_TILES], F32, kind="ExternalInput")
    nkey_d = nc.dram_tensor("nkey", [128, N_TILES], F32, kind="ExternalInput")
    iota_d = nc.dram_tensor("iota", [128, W], F16, kind="ExternalInput")
    ohpre_d = nc.dram_tensor(
        "ohpre", [128, _PRE_TOT * W], F8, kind="ExternalInput"
    )
    out_d = nc.dram_tensor("out", [128, W], F32, kind="ExternalOutput")

    with tile.TileContext(nc) as tc:
        with (
            tc.tile_pool(name="io", bufs=5) as pio,
            tc.tile_pool(name="pre", bufs=3) as ppre,
            tc.tile_pool(name="oh", bufs=24) as poh,
            tc.tile_pool(name="tmp", bufs=3) as ptmp,
            tc.tile_pool(name="const", bufs=1) as pconst,
            tc.tile_pool(name="psum", bufs=1, space="PSUM") as pps,
        ):
            iota_sb = pconst.tile([128, W], F16)
            nc.sync.dma_start(iota_sb[:], iota_d[:])
            key_sb = pconst.tile([128, N_TILES], F32)
            nc.sync.dma_start(key_sb[:], key_d[:])
            nkey_sb = pconst.tile([128, N_TILES], F32)
            nc.sync.dma_start(nkey_sb[:], nkey_d[:])
            acc = pps.tile([128, W], F32)

            t_idx = 0
            row = 0
            pre_off = 0
            for ct, kp, ka in _CHUNKS:
                npts = ct * 128
                if kp > 0:
                    pre = ppre.tile([128, kp * W], F8, tag="pre")
                    nc.sync.dma_start(
                        pre[:], ohpre_d[:, pre_off * W : (pre_off + kp) * W]
                    )
                ph = pio.tile([128, npts], F16, tag="ph")
                src = pred_d[row : row + npts, :].rearrange(
                    "(p j) d -> p (j d)", p=128
                )
                nc.sync.dma_start(ph[:], src)
                srcs = _interleave(ct, kp, ka)
                pre_used = 0
                for j in range(ct):
                    if srcs[j] == "pre":
                        rhs = pre[:, pre_used * W : (pre_used + 1) * W]
                        pre_used += 1
                    elif srcs[j] == "act":
                        # ScalarE path: oh = relu(1 - |iota - key|), exact
                        oh = poh.tile([128, W], F16)
                        tmp = ptmp.tile([128, W], F16)
                        nc.scalar.activation(
                            tmp[:], iota_sb[:], AF.Abs,
                            bias=nkey_sb[:, t_idx : t_idx + 1], scale=1.0,
                        )
                        nc.scalar.activation(
                            oh[:], tmp[:], AF.Relu, bias=1.0, scale=-1.0,
                        )
                        rhs = oh[:]
                    else:
                        oh = poh.tile([128, W], F16)
                        nc.vector.tensor_scalar(
                            oh[:],
                            iota_sb[:],
                            key_sb[:, t_idx : t_idx + 1],
                            None,
                            mybir.AluOpType.is_equal,
                        )
                        rhs = oh[:]
                    nc.tensor.matmul(
                        acc[:],
                        ph[:, j * 128 : (j + 1) * 128],
                        rhs,
                        start=(t_idx == 0),
                        stop=(t_idx == N_TILES - 1),
                    )
                    t_idx += 1
                pre_off += kp
                row += npts
            out_sb = pconst.tile([128, W], F32)
            nc.vector.tensor_copy(out_sb[:], acc[:])
            nc.sync.dma_start(out_d[:], out_sb[:])
    nc.compile()
    return nc


_NC = None


def _get_nc():
    global _NC
    if _NC is None:
        _NC = _build_nc()
    return _NC


def _key_layout(key_flat: np.ndarray) -> np.ndarray:
    """[SHARD] f32 -> [128, N_TILES] f32 matching the kernel's point order:
    within a chunk of `ct` groups starting at flat row `row`, partition p,
    column j holds point row + p*ct + j."""
    cols = []
    row = 0
    for ct, _, _ in _CHUNKS:
        cols.append(key_flat[row : row + ct * 128].reshape(128, ct))
        row += ct * 128
    return np.ascontiguousarray(np.concatenate(cols, axis=1))


_PRE_IDX = []
_t = 0
for _ct, _kp, _ka in _CHUNKS:
    _srcs = _interleave(_ct, _kp, _ka)
    _PRE_IDX.extend(_t + _j for _j in range(_ct) if _srcs[_j] == "pre")
    _t += _ct


def _host_ohpre(key2d: np.ndarray) -> np.ndarray:
    import ml_dtypes

    sel = key2d[:, _PRE_IDX]                     # [128, PRE_TOT]
    oh = sel[:, :, None] == np.arange(W, dtype=np.float32)[None, None, :]
    return np.ascontiguousarray(
        oh.astype(ml_dtypes.float8_e4m3).reshape(128, _PRE_TOT * W)
    )


def _prep_in_maps(pred, key_full):
    iota = np.tile(np.arange(W, dtype=np.float16), (128, 1))
    in_maps = []
    for i in range(N_CORES):
        s = _STARTS[i]
        k = key_full[s : s + SHARD].copy()
        own_lo, own_hi = i * OWN, (i + 1) * OWN
        gidx = np.arange(s, s + SHARD)
        k[(gidx < own_lo) | (gidx >= own_hi)] = INVALID
        k2 = _key_layout(k)
        in_maps.append(
            {
                "pred": np.ascontiguousarray(
                    pred[s : s + SHARD], dtype=np.float16
                ),
                "key": k2,
                "nkey": -k2,
                "iota": iota,
                "ohpre": _host_ohpre(k2),
            }
        )
    return in_maps


def _make_keys(seg, grp, vm):
    valid = (vm > 0) & (seg != -1)
    segc = np.clip(seg, 0, C - 1)
    in_group = (grp == 0) | (grp == 1)
    key_full = np.where(
        valid & in_group, segc + BOFF * grp, int(INVALID)
    ).astype(np.float32)
    return key_full, valid, segc


def kernel(pred, target, valid_feat_mask, segment, group_assign):
    pred = np.asarray(pred, dtype=np.float32)
    seg = np.asarray(segment).astype(np.int64)
    grp = np.asarray(group_assign).astype(np.int64)
    vm = np.asarray(valid_feat_mask)

    key_full, valid, segc = _make_keys(seg, grp, vm)
    in_maps = _prep_in_maps(pred, key_full)

    nc = _get_nc()
    res = run_bass_kernel_spmd(nc, in_maps, core_ids=list(range(N_CORES)))

    total = np.zeros((128, W), np.float64)
    for r in res.results:
        total += r["out"].astype(np.float64)
    sum_a = total[:, 0:C].T          # [C, D]
    sum_b = total[:, BOFF : BOFF + C].T

    ga = valid & (grp == 0)
    gb = valid & (grp == 1)
    cnt_a = np.bincount(segc[ga], minlength=C).astype(np.float64)
    cnt_b = np.bincount(segc[gb], minlength=C).astype(np.float64)

    mean_a = sum_a / np.maximum(cnt_a, 1.0)[:, None]
    mean_b = sum_b / np.maximum(cnt_b, 1.0)[:, None]
    a = mean_a / np.linalg.norm(mean_a, axis=1, keepdims=True)
    b = mean_b / np.linalg.norm(mean_b, axis=1, keepdims=True)
    logits = (a @ b.T) / TEMPERATURE
    diag = np.diagonal(logits)

    def lse(x, axis):
        m = x.max(axis=axis)
        return m + np.log(np.exp(x - np.expand_dims(m, axis)).sum(axis=axis))

    loss_a = np.mean(lse(logits, 1) - diag)
    loss_b = np.mean(lse(logits, 0) - diag)
    loss = LOSS_WEIGHT * (loss_a + loss_b) / 2.0
    return np.asarray(loss, dtype=np.float32)


# revision 15
# speedup vs baseline: 1.3804x; 1.0043x over previous
"""AggregatedContrastiveLoss on 8 Trainium2 NeuronCores.

Strategy (data-parallel over the N=2M points dimension):
  - Each of 8 cores streams a ~250k-point shard of pred (as fp16) from HBM.
  - Per 128-point group, TensorE accumulates predT @ onehot[128, 304] into
    PSUM [128d, 304] — the per-(class, group) feature sums for group A
    (cols 0..149) and group B (cols 152..301), where the one-hot selection
    matrix comes from a host-packed key (key = seg + 152*group, or an
    out-of-range value for masked/overlap points).
  - One-hot groups are produced by three sources in parallel, balanced so
    DMA, VectorE and ScalarE all saturate together:
      * k_pre groups/chunk: precomputed fp16 one-hots DMAd from HBM
      * middle groups: VectorE tensor_scalar is_equal vs an iota row
      * k_act groups/chunk: ScalarE relu(1 - |iota - key|) (exact)
  - Host reduces the 8 partial [128,304] outputs, computes per-class counts
    from the (tiny) int arrays, and runs the [150,128]-level normalize +
    [150,150] InfoNCE finalize in float64.
The `target` input is unused by the loss math and never transferred.
"""
import numpy as np

import concourse.bacc as bacc
import concourse.mybir as mybir
import concourse.tile as tile
from concourse.bass_utils import run_bass_kernel_spmd

F32 = mybir.dt.float32
F16 = mybir.dt.float16
F8 = mybir.dt.float8e4
AF = mybir.ActivationFunctionType

N = 2_000_000
D = 128
C = 150
TEMPERATURE = 0.2
LOSS_WEIGHT = 1.0

N_CORES = 8
OWN = N // N_CORES            # 250_000 points owned per core
SHARD = 250_112               # 1954 groups of 128 (>= OWN, multiple of 128)
N_TILES = SHARD // 128        # 1954
CHUNK_TILES = 64              # 8192 points (2 MB fp16) per pred DMA
K_PRE = 17                    # groups/chunk with precomputed one-hot (DMA)
K_ACT = 8                     # groups/chunk built on ScalarE
W = 304                       # one-hot width (A: 0..149, B: 152..301)
BOFF = 152
INVALID = 1000.0

_STARTS = [min(i * OWN, N - SHARD) for i in range(N_CORES)]


def _plan_chunks():
    chunks = []
    rem = N_TILES
    first = [8, 8, 16, 32]    # priming chunks fill the pipeline quickly
    for ct in first:
        chunks.append(ct)
        rem -= ct
    while rem > 0:
        chunks.append(min(CHUNK_TILES, rem))
        rem -= chunks[-1]
    return [
        (ct, (ct * K_PRE) // CHUNK_TILES, (ct * K_ACT) // CHUNK_TILES)
        for ct in chunks
    ]


_CHUNKS = _plan_chunks()
_PRE_TOT = sum(kp for _, kp, _ in _CHUNKS)


def _interleave(ct, kp, ka):
    """Spread the three one-hot sources evenly across a chunk's groups so
    PE's in-order PSUM consumption sees a homogeneous production mix."""
    srcs = []
    acc_p = acc_a = 0.0
    for j in range(ct):
        acc_p += kp / ct
        acc_a += ka / ct
        if acc_p >= 1.0:
            srcs.append("pre")
            acc_p -= 1.0
        elif acc_a >= 1.0:
            srcs.append("act")
            acc_a -= 1.0
        else:
            srcs.append("dve")
    # fix rounding drift
    while srcs.count("pre") < kp:
        srcs[srcs.index("dve")] = "pre"
    while srcs.count("act") < ka:
        srcs[len(srcs) - 1 - srcs[::-1].index("dve")] = "act"
    assert srcs.count("pre") == kp and srcs.count("act") == ka
    return srcs


def _build_nc():
    nc = bacc.Bacc(
        "TRN2", target_bir_lowering=False, debug=False, num_devices=N_CORES
    )
    pred_d = nc.dram_tensor("pred", [SHARD, D], F16, kind="ExternalInput")
    key_d = nc.dram_tensor("key", [128, N_TILES], F32, kind="ExternalInput")
    iota_d = nc.dram_tensor("iota", [128, W], F16, kind="ExternalInput")
    ohpre_d = nc.dram_tensor(
        "ohpre", [128, _PRE_TOT * W], F8, kind="ExternalInput"
    )
    out_d = nc.dram_tensor("out", [128, W], F32, kind="ExternalOutput")

    with tile.TileContext(nc) as tc:
        with (
            tc.tile_pool(name="io", bufs=6) as pio,
            tc.tile_pool(name="pre", bufs=4) as ppre,
            tc.tile_pool(name="oh", bufs=32) as poh,
            tc.tile_pool(name="tmp", bufs=3) as ptmp,
            tc.tile_pool(name="const", bufs=1) as pconst,
            tc.tile_pool(name="psum", bufs=1, space="PSUM") as pps,
        ):
            iota_sb = pconst.tile([128, W], F16)
            nc.sync.dma_start(iota_sb[:], iota_d[:])
            key_sb = pconst.tile([128, N_TILES], F32)
            nc.sync.dma_start(key_sb[:], key_d[:])
            nkey_sb = pconst.tile([128, N_TILES], F32)
            nc.vector.tensor_scalar(
                nkey_sb[:], key_sb[:], -1.0, None, mybir.AluOpType.mult
            )
            acc = pps.tile([128, W], F32)

            t_idx = 0
            row = 0
            pre_off = 0
            for ct, kp, ka in _CHUNKS:
                npts = ct * 128
                if kp > 0:
                    pre = ppre.tile([128, kp * W], F8, tag="pre")
                    nc.sync.dma_start(
                        pre[:], ohpre_d[:, pre_off * W : (pre_off + kp) * W]
                    )
                ph = pio.tile([128, npts], F16, tag="ph")
                src = pred_d[row : row + npts, :].rearrange(
                    "(p j) d -> p (j d)", p=128
                )
                nc.sync.dma_start(ph[:], src)
                srcs = _interleave(ct, kp, ka)
                pre_used = 0
                for j in range(ct):
                    if srcs[j] == "pre":
                        rhs = pre[:, pre_used * W : (pre_used + 1) * W]
                        pre_used += 1
                    elif srcs[j] == "act":
                        # ScalarE path: oh = relu(1 - |iota - key|), exact
                        oh = poh.tile([128, W], F16)
                        tmp = ptmp.tile([128, W], F16)
                        nc.scalar.activation(
                            tmp[:], iota_sb[:], AF.Abs,
                            bias=nkey_sb[:, t_idx : t_idx + 1], scale=1.0,
                        )
                        nc.scalar.activation(
                            oh[:], tmp[:], AF.Relu, bias=1.0, scale=-1.0,
                        )
                        rhs = oh[:]
                    else:
                        oh = poh.tile([128, W], F16)
                        nc.vector.tensor_scalar(
                            oh[:],
                            iota_sb[:],
                            key_sb[:, t_idx : t_idx + 1],
                            None,
                            mybir.AluOpType.is_equal,
                        )
                        rhs = oh[:]
                    nc.tensor.matmul(
                        acc[:],
                        ph[:, j * 128 : (j + 1) * 128],
                        rhs,
                        start=(t_idx == 0),
                        stop=(t_idx == N_TILES - 1),
                    )
                    t_idx += 1
                pre_off += kp
                row += npts
            out_sb = pconst.tile([128, W], F32)
            nc.vector.tensor_copy(out_sb[:], acc[:])
            nc.sync.dma_start(out_d[:], out_sb[:])
    nc.compile()
    return nc


_NC = None


def _get_nc():
    global _NC
    if _NC is None:
        _NC = _build_nc()
    return _NC


def _key_layout(key_flat: np.ndarray) -> np.ndarray:
    """[SHARD] f32 -> [128, N_TILES] f32 matching the kernel's point order:
    within a chunk of `ct` groups starting at flat row `row`, partition p,
    column j holds point row + p*ct + j."""
    cols = []
    row = 0
    for ct, _, _ in _CHUNKS:
        cols.append(key_flat[row : row + ct * 128].reshape(128, ct))
        row += ct * 128
    return np.ascontiguousarray(np.concatenate(cols, axis=1))


_PRE_IDX = []
_t = 0
for _ct, _kp, _ka in _CHUNKS:
    _srcs = _interleave(_ct, _kp, _ka)
    _PRE_IDX.extend(_t + _j for _j in range(_ct) if _srcs[_j] == "pre")
    _t += _ct


def _host_ohpre(key2d: np.ndarray) -> np.ndarray:
    import ml_dtypes

    sel = key2d[:, _PRE_IDX]                     # [128, PRE_TOT]
    oh = sel[:, :, None] == np.arange(W, dtype=np.float32)[None, None, :]
    return np.ascontiguousarray(
        oh.astype(ml_dtypes.float8_e4m3).reshape(128, _PRE_TOT * W)
    )


def _prep_in_maps(pred, key_full):
    iota = np.tile(np.arange(W, dtype=np.float16), (128, 1))
    in_maps = []
    for i in range(N_CORES):
        s = _STARTS[i]
        k = key_full[s : s + SHARD].copy()
        own_lo, own_hi = i * OWN, (i + 1) * OWN
        gidx = np.arange(s, s + SHARD)
        k[(gidx < own_lo) | (gidx >= own_hi)] = INVALID
        k2 = _key_layout(k)
        in_maps.append(
            {
                "pred": np.ascontiguousarray(
                    pred[s : s + SHARD], dtype=np.float16
                ),
                "key": k2,
                "iota": iota,
                "ohpre": _host_ohpre(k2),
            }
        )
    return in_maps


def _make_keys(seg, grp, vm):
    valid = (vm > 0) & (seg != -1)
    segc = np.clip(seg, 0, C - 1)
    in_group = (grp == 0) | (grp == 1)
    key_full = np.where(
        valid & in_group, segc + BOFF * grp, int(INVALID)
    ).astype(np.float32)
    return key_full, valid, segc


def kernel(pred, target, valid_feat_mask, segment, group_assign):
    pred = np.asarray(pred, dtype=np.float32)
    seg = np.asarray(segment).astype(np.int64)
    grp = np.asarray(group_assign).astype(np.int64)
    vm = np.asarray(valid_feat_mask)

    key_full, valid, segc = _make_keys(seg, grp, vm)
    in_maps = _prep_in_maps(pred, key_full)

    nc = _get_nc()
    res = run_bass_kernel_spmd(nc, in_maps, core_ids=list(range(N_CORES)))

    total = np.zeros((128, W), np.float64)
    for r in res.results:
        total += r["out"].astype(np.float64)
    sum_a = total[:, 0:C].T          # [C, D]
    sum_b = total[:, BOFF : BOFF + C].T

    ga = valid & (grp == 0)
    gb = valid & (grp == 1)
    cnt_a = np.bincount(segc[ga], minlength=C).astype(np.float64)
    cnt_b = np.bincount(segc[gb], minlength=C).astype(np.float64)

    mean_a = sum_a / np.maximum(cnt_a, 1.0)[:, None]
    mean_b = sum_b / np.maximum(cnt_b, 1.0)[:, None]
    a = mean_a / np.linalg.norm(mean_a, axis=1, keepdims=True)
    b = mean_b / np.linalg.norm(mean_b, axis=1, keepdims=True)
    logits = (a @ b.T) / TEMPERATURE
    diag = np.diagonal(logits)

    def lse(x, axis):
        m = x.max(axis=axis)
        return m + np.log(np.exp(x - np.expand_dims(m, axis)).sum(axis=axis))

    loss_a = np.mean(lse(logits, 1) - diag)
    loss_b = np.mean(lse(logits, 0) - diag)
    loss = LOSS_WEIGHT * (loss_a + loss_b) / 2.0
    return np.asarray(loss, dtype=np.float32)
